# revision 9
# baseline (speedup 1.0000x reference)
"""Distributed causal attention (RoPE) kernel for 8 TRN2 NeuronCores.

Problem: B=4, S=2048, dim=2048, H=16 heads, D=128 head dim.
  q,k,v = x @ W{q,k,v}.T (heads), RoPE(q,k), causal softmax(q k^T/sqrt(D)) v,
  out = concat_heads @ Wo.T

Sharding: tensor-parallel over heads — 2 heads per core. Each core:
  - computes qT/kT [d, t] and v [s, e] for its 2 heads (weights pre-transposed
    host-side so every matmul operand is in its natural layout),
  - attention in "scoresT" orientation [key s on partitions, query t free]:
    exp without max-subtraction (bf16 holds e^16 fine); the softmax
    denominator comes from accumulating exp tiles on DVE (bf16 adds) and ONE
    all-ones [128,128] stationary matmul per query tile,
  - per-(batch,head) All-to-All (DRAM->DRAM) reshards attention output from
    head-shard to row-shard,
  - row-local output projection; host reassembles the row shards.

Schedule (v2): PE streams at ~0.5ns/col on this part, so the only wins are
scheduling. Warmup matmuls flip the HAM clock gate before real work arrives.
Out-projections for batches 0,1,3 are DEFERRED to an f-major tail (~100us of
matmul) that covers the last two collectives; only out_proj(b2) stays
interleaved with batch-3 attention as TensorE filler for the ACT-bound exp
chain. The ACT queue carries nothing but exps steady-state (weight/wo/at DMA
triggers live on scalar only at start/tail; x/cos/sin/ot on sync; collectives
and output stores on gpsimd). Output is written bf16 (err budget allows) to
halve store traffic.
"""

import numpy as np
import ml_dtypes

B, S, DIM = 4, 2048, 2048
H, D = 16, 128
NCORES = 8
HPC = H // NCORES            # heads per core = 2
E = HPC * D                  # per-core inner width = 256
BS = B * S                   # 8192 flattened rows
KT = DIM // 128              # 16 contraction tiles
TQ = 512                     # query tile width
NQ = S // TQ                 # 4 query tiles per (b,h)
NB = S // TQ                 # 4 x-blocks per batch
RPB = S // NCORES            # 256 output rows per core per batch
ROWS = B * RPB               # 1024 output rows per core
SCALE = 1.0 / np.sqrt(D)
WARM = 32                    # HAM warmup matmuls (N=128, ~3.5us: flips 4/8->8/8)

_CACHE = {}


def _build(causal: bool):
    from concourse import bacc, tile, mybir

    f32 = mybir.dt.float32
    bf16 = mybir.dt.bfloat16
    Exp = mybir.ActivationFunctionType.Exp

    nc = bacc.Bacc(None, target_bir_lowering=False, num_devices=NCORES)

    # host layouts: xT pre-tiled [block n, ktile, 128, 512]
    xT_d = nc.dram_tensor("xT", [B * NB, KT, 128, TQ], bf16, kind="ExternalInput")
    wq_d = nc.dram_tensor("wqT", [KT, 128, E], bf16, kind="ExternalInput")
    wk_d = nc.dram_tensor("wkT", [KT, 128, E], bf16, kind="ExternalInput")
    wv_d = nc.dram_tensor("wvT", [KT, 128, E], bf16, kind="ExternalInput")
    wo_d = nc.dram_tensor("woT", [DIM // TQ, KT, 128, TQ], bf16, kind="ExternalInput")
    cos_d = nc.dram_tensor("cosT", [128, BS], bf16, kind="ExternalInput")
    sin_d = nc.dram_tensor("sinT", [128, BS], bf16, kind="ExternalInput")
    msk_d = nc.dram_tensor("masks", [128, TQ], bf16, kind="ExternalInput")
    out_d = nc.dram_tensor("out", [ROWS, DIM], bf16, kind="ExternalOutput")

    with tile.TileContext(nc) as tc:
        with (
            tc.tile_pool(name="const", bufs=1) as constp,
            tc.tile_pool(name="dram", bufs=1, space="DRAM") as dramp,
        ):
            a2a_in = [[dramp.tile([NCORES, 128, RPB], bf16, name=f"a2ai{b}h{h}")
                       for h in range(HPC)] for b in range(B)]
            a2a_out = [[dramp.tile([NCORES, 128, RPB], bf16, name=f"a2ao{b}h{h}")
                        for h in range(HPC)] for b in range(B)]

            ones_col = constp.tile([128, 128], bf16)
            nc.gpsimd.memset(ones_col[:], 1.0)

            # startup DMAs on the scalar ring (free until first exp), finest
            # first so the first projection matmul can begin after ~2 chunks.
            wq_sb = constp.tile([128, KT, E], bf16)
            wk_sb = constp.tile([128, KT, E], bf16)
            wv_sb = constp.tile([128, KT, E], bf16)
            for k in range(0, KT, 2):
                nc.scalar.dma_start(wq_sb[:, k:k + 2, :],
                                    wq_d[k:k + 2].rearrange("k p e -> p k e"))
            for k in range(0, KT, 2):
                nc.scalar.dma_start(wk_sb[:, k:k + 2, :],
                                    wk_d[k:k + 2].rearrange("k p e -> p k e"))
            for k in range(0, KT, 2):
                nc.scalar.dma_start(wv_sb[:, k:k + 2, :],
                                    wv_d[k:k + 2].rearrange("k p e -> p k e"))
            if causal:
                msk_sb = constp.tile([128, TQ], bf16)
                nc.gpsimd.dma_start(msk_sb[:], msk_d[:])

            with (
                tc.tile_pool(name="qkv", bufs=2) as qkvp,
                tc.tile_pool(name="xblk", bufs=2) as xp,
                tc.tile_pool(name="cs", bufs=2) as cp,
                tc.tile_pool(name="rope", bufs=2) as rp,
                tc.tile_pool(name="att", bufs=4) as ap,
                tc.tile_pool(name="ex", bufs=6) as exp_pool,
                tc.tile_pool(name="exa", bufs=2) as exap,
                tc.tile_pool(name="wo", bufs=2) as wop,
                tc.tile_pool(name="attin", bufs=4) as atp,
                tc.tile_pool(name="res", bufs=4) as resp,
                tc.tile_pool(name="ps1", bufs=1, space="PSUM") as pp1,
                tc.tile_pool(name="ps2", bufs=1, space="PSUM") as pp2,
            ):
                # HAM warmup: throwaway matmuls so the clock gate is at 8/8
                # by the time the first data-dependent matmul issues.
                for _ in range(WARM):
                    wps = pp1.tile([128, TQ], f32, tag="qk", bufs=2)
                    nc.tensor.matmul(wps[:, 0:128], ones_col[:], ones_col[:],
                                     start=True, stop=True)

                def p1(b):
                    """Projections + RoPE for batch b -> qb, kb, vb."""
                    qb = qkvp.tile([128, HPC, S], bf16, tag="q", name=f"q{b}")
                    kb = qkvp.tile([128, HPC, S], bf16, tag="k", name=f"k{b}")
                    vb = qkvp.tile([128, S // 128, E], bf16, tag="v",
                                   name=f"v{b}")
                    for n in range(NB):
                        c0 = n * TQ
                        g0 = b * S + c0
                        xblk = xp.tile([128, KT, TQ], bf16, tag="xblk")
                        cos_b = cp.tile([128, TQ], bf16, tag="cos")
                        sin_b = cp.tile([128, TQ], bf16, tag="sin")
                        nc.sync.dma_start(cos_b[:], cos_d[:, g0:g0 + TQ])
                        nc.sync.dma_start(sin_b[:], sin_d[:, g0:g0 + TQ])
                        if b == 0 and n == 0:
                            # finest-grained first load so matmuls start ASAP
                            for k in range(KT):
                                nc.sync.dma_start(xblk[:, k, :], xT_d[0, k])
                        else:
                            # alternate queues by block parity so consecutive
                            # blocks stream concurrently
                            xeng = nc.sync if n % 2 == 0 else nc.gpsimd
                            for k in range(0, KT, 8):
                                xeng.dma_start(
                                    xblk[:, k:k + 8, :],
                                    xT_d[b * NB + n, k:k + 8].rearrange(
                                        "k p t -> p k t"))

                        for w_sb, dst in ((wq_sb, qb), (wk_sb, kb)):
                            for h in range(HPC):
                                ps = pp1.tile([128, TQ], f32, tag="qk", bufs=2)
                                for k in range(KT):
                                    nc.tensor.matmul(
                                        ps[:],
                                        w_sb[:, k, h * 128:(h + 1) * 128],
                                        xblk[:, k, :],
                                        start=(k == 0), stop=(k == KT - 1),
                                    )
                                t0_ = rp.tile([128, TQ], f32, tag="t0", bufs=2)
                                nc.vector.tensor_mul(t0_[0:64, :],
                                                     ps[64:128, :],
                                                     sin_b[0:64, :])
                                nc.vector.tensor_mul(t0_[64:128, :],
                                                     ps[0:64, :],
                                                     sin_b[64:128, :])
                                t1_ = rp.tile([128, TQ], f32, tag="t1", bufs=2)
                                nc.vector.tensor_mul(t1_[:], ps[:], cos_b[:])
                                nc.vector.tensor_add(dst[:, h, c0:c0 + TQ],
                                                     t0_[:], t1_[:])

                        for ss in range(TQ // 128):
                            vps = pp1.tile([128, E], f32, tag="v", bufs=1)
                            for k in range(KT):
                                nc.tensor.matmul(
                                    vps[:],
                                    xblk[:, k, ss * 128:(ss + 1) * 128],
                                    wv_sb[:, k, :],
                                    start=(k == 0), stop=(k == KT - 1),
                                )
                            nc.vector.tensor_copy(vb[:, n * 4 + ss, :], vps[:])
                    return qb, kb, vb

                def attention(b, h, qb, kb, vb, weave=None):
                    """Attention for (batch b, local head h) -> a2a_in[b][h].
                    Returns the per-qtile ot tiles (used as scheduling
                    tokens).  weave(tq, j) is called after each j-step to
                    emit a few out-projection matmuls between attention
                    matmuls — keeps TensorE dense without letting 16-matmul
                    blocks starve the ACT exp chain."""
                    ots = []
                    for tq in range(NQ):
                        t0 = tq * TQ
                        jmax = (tq + 1) * (TQ // 128) if causal else S // 128
                        av = pp2.tile([128, TQ], f32, tag="av", bufs=2)
                        exa = exap.tile([128, TQ], bf16, tag="exa", bufs=2)
                        for j in range(jmax):
                            s0 = j * 128
                            # diagonal tiles: only queries t >= s attend;
                            # compute the trapezoid [off:TQ) at reduced width
                            diag = causal and j >= jmax - 4
                            off = 128 * (j - (jmax - 4)) if diag else 0
                            w = TQ - off
                            # sc ring depth 3 (cs shares it): ACT can run up
                            # to 3 exp tiles ahead, riding out op-matmul
                            # bursts on TensorE
                            sc = pp2.tile([128, TQ], f32, tag="sc", bufs=3)
                            nc.tensor.matmul(
                                sc[:, 0:w], kb[:, h, s0:s0 + 128],
                                qb[:, h, t0 + off:t0 + TQ],
                                start=True, stop=True,
                            )
                            ex = exp_pool.tile([128, TQ], bf16, tag="ex")
                            nc.scalar.activation(ex[:, 0:w], sc[:, 0:w], Exp,
                                                 scale=float(SCALE))
                            if diag:
                                exm = exp_pool.tile([128, TQ], bf16, tag="exm",
                                                    bufs=2)
                                nc.vector.tensor_mul(exm[:, 0:w], ex[:, 0:w],
                                                     msk_sb[:, 0:w])
                                ex = exm
                            nc.tensor.matmul(
                                av[:, off:TQ], vb[:, j, h * 128:(h + 1) * 128],
                                ex[:, 0:w],
                                start=(j == 0), stop=(j == jmax - 1),
                            )
                            # softmax denominator: accumulate exp tiles in
                            # bf16 on DVE; one ones-matmul per query tile
                            if j == 0:
                                nc.vector.tensor_copy(exa[:], ex[:])
                            else:
                                nc.vector.tensor_add(exa[:, off:TQ],
                                                     exa[:, off:TQ],
                                                     ex[:, 0:w])
                            if weave is not None:
                                weave(tq, j)
                        cs = pp2.tile([128, TQ], f32, tag="sc", bufs=3)
                        nc.tensor.matmul(cs[:], ones_col[:], exa[:],
                                         start=True, stop=True)
                        # 1/colsum: approx reciprocal (~18 bits, 1 DVE op)
                        rec = ap.tile([128, TQ], f32, tag="rec", bufs=2)
                        nc.vector.reciprocal_approx_fast(rec[:], cs[:])
                        ot = ap.tile([128, TQ], bf16, tag="ot", bufs=3)
                        nc.vector.tensor_mul(ot[:], av[:], rec[:])
                        # queries [t0, t0+512) of batch b go to dest cores
                        # 2tq (first 256) and 2tq+1 (second 256)
                        nc.sync.dma_start(a2a_in[b][h][2 * tq], ot[:, 0:RPB])
                        nc.sync.dma_start(a2a_in[b][h][2 * tq + 1],
                                          ot[:, RPB:2 * RPB])
                        ots.append(ot)
                    return ots

                def a2a(b, h):
                    nc.gpsimd.collective_compute(
                        "AllToAll", mybir.AluOpType.bypass,
                        replica_groups=[list(range(NCORES))],
                        ins=[a2a_in[b][h][:].opt()],
                        outs=[a2a_out[b][h][:].opt()],
                    )

                def load_at(b, eng, dep=None, heads=(0, 1)):
                    """Load the resharded attention rows of batch b into SBUF.
                    dep (a tile) token-binds the loads so the scheduler cannot
                    place them (or dependent matmuls) ahead of work that must
                    come first."""
                    at_sb = atp.tile([128, KT, RPB], bf16, tag="at",
                                     name=f"at{b}")
                    if dep is not None:
                        nc.vector.tensor_copy(at_sb[0:1, :, 0:1],
                                              dep[0:1, 0:KT])
                    for h in heads:
                        eng.dma_start(
                            at_sb[:, h:KT:2, :],
                            a2a_out[b][h].rearrange("i p r -> p i r"))
                    return at_sb

                def op_group(b, at_sb, wo_f, f, tt, store_eng):
                    """One out-projection group: rows [tt*128,(tt+1)*128) of
                    this core's batch-b shard x output cols [f*TQ,(f+1)*TQ)."""
                    ops = pp1.tile([128, TQ], f32, tag="qk", bufs=2)
                    for ki in range(KT):
                        nc.tensor.matmul(
                            ops[:],
                            at_sb[:, ki, tt * 128:(tt + 1) * 128],
                            wo_f[:, ki, :],
                            start=(ki == 0), stop=(ki == KT - 1),
                        )
                    res = resp.tile([128, TQ], bf16, tag="res")
                    nc.vector.tensor_copy(res[:], ops[:])
                    store_eng.dma_start(
                        out_d[b * RPB + tt * 128:b * RPB + (tt + 1) * 128,
                              f * TQ:(f + 1) * TQ],
                        res[:])
                    return res

                wo_tiles = {}

                class OpWeaver:
                    """Emits one batch's out-projection a few matmuls at a
                    time so it interleaves with attention at j-granularity
                    instead of 16-matmul blocks."""

                    def __init__(self, b, at_sb):
                        self.b = b
                        self.at = at_sb
                        self.jobs = [(f, tt) for f in range(DIM // TQ)
                                     for tt in range(RPB // 128)]
                        self.ji = 0
                        self.ki = 0
                        self.ps = None

                    def step(self, n):
                        for _ in range(n):
                            if self.ji >= len(self.jobs):
                                return
                            f, tt = self.jobs[self.ji]
                            if self.ki == 0:
                                if f not in wo_tiles:
                                    wo_f = wop.tile([128, KT, TQ], bf16,
                                                    tag="wo", bufs=2)
                                    nc.gpsimd.dma_start(
                                        wo_f[:],
                                        wo_d[f].rearrange("k p t -> p k t"))
                                    wo_tiles[f] = wo_f
                                self.ps = pp1.tile([128, TQ], f32, tag="qk",
                                                   bufs=2)
                            nc.tensor.matmul(
                                self.ps[:],
                                self.at[:, self.ki, tt * 128:(tt + 1) * 128],
                                wo_tiles[f][:, self.ki, :],
                                start=(self.ki == 0), stop=(self.ki == KT - 1),
                            )
                            self.ki += 1
                            if self.ki == KT:
                                self.ki = 0
                                self.ji += 1
                                res = resp.tile([128, TQ], bf16, tag="res")
                                nc.vector.tensor_copy(res[:], self.ps[:])
                                nc.gpsimd.dma_start(
                                    out_d[self.b * RPB + tt * 128:
                                          self.b * RPB + (tt + 1) * 128,
                                          f * TQ:(f + 1) * TQ],
                                    res[:])

                    def finish(self):
                        self.step(1 << 30)

                # ---- main pipeline ----
                prev_ot = None
                for b in range(B):
                    qb, kb, vb = p1(b)
                    if b < B - 1:
                        ot0 = attention(b, 0, qb, kb, vb)
                        a2a(b, 0)
                        prev_ot = attention(b, 1, qb, kb, vb)
                        a2a(b, 1)
                    else:
                        # out_proj(b2) woven into batch-3 attention at
                        # j-granularity: fills the ACT-bound exp chain's
                        # TensorE bubbles without starving it, and keeps the
                        # last A2A triggers early.  at2 token-bound to the end
                        # of attn(2,1) (its real dep, A2A(2,1), lands during
                        # P1(3)).
                        at2 = load_at(B - 2, nc.gpsimd, dep=prev_ot[3])
                        weaver = OpWeaver(B - 2, at2)
                        ot0 = attention(
                            b, 0, qb, kb, vb,
                            weave=lambda tq, j: weaver.step(2) if tq >= 2
                            else None)
                        a2a(b, 0)
                        ot1 = attention(b, 1, qb, kb, vb,
                                        weave=lambda tq, j: weaver.step(2))
                        a2a(b, 1)
                        weaver.finish()

                # ---- tail: deferred out-projections cover the last two
                # collectives (~100us of matmul vs ~65us of collective) ----
                at_tiles = {0: load_at(0, nc.gpsimd),
                            1: load_at(1, nc.gpsimd)}
                mid_res = None
                for f in range(DIM // TQ):
                    wo_f = wop.tile([128, KT, TQ], bf16, tag="wo", bufs=2)
                    nc.scalar.dma_start(wo_f[:],
                                        wo_d[f].rearrange("k p t -> p k t"))
                    wo_tiles[f] = wo_f
                    for b2 in (0, 1):
                        for tt in range(RPB // 128):
                            eng = nc.sync if (b2 + tt) % 2 == 0 else nc.gpsimd
                            r = op_group(b2, at_tiles[b2], wo_f, f, tt, eng)
                            if f == 1 and b2 == 1 and tt == 1:
                                mid_res = r
                # batch 3: wo f2/f3 still resident from phase 1; f0/f1
                # re-stream behind them.  at3 token-bound to mid-phase-1 so
                # its matmuls strictly follow most of the cover work; head
                # halves load separately (head 0's collective lands ~30us
                # before head 1's).
                at3 = load_at(3, nc.scalar, dep=mid_res, heads=(0,))
                nc.scalar.dma_start(
                    at3[:, 1:KT:2, :],
                    a2a_out[3][1].rearrange("i p r -> p i r"))
                for f in (2, 3, 0, 1):
                    if f >= 2:
                        wo_f = wo_tiles[f]
                    else:
                        wo_f = wop.tile([128, KT, TQ], bf16, tag="wo", bufs=2)
                        nc.scalar.dma_start(
                            wo_f[:], wo_d[f].rearrange("k p t -> p k t"))
                    for tt in range(RPB // 128):
                        eng = nc.sync if tt % 2 == 0 else nc.gpsimd
                        op_group(3, at3, wo_f, f, tt, eng)

    nc.compile()
    return nc


def _prep_inputs(x, Wq, Wk, Wv, Wo, causal):
    bf16 = ml_dtypes.bfloat16
    xT = np.ascontiguousarray(x.reshape(BS, DIM).T).astype(bf16)  # [dim, BS]
    # pre-tile: [block n, ktile, 128, 512]
    xTt = np.ascontiguousarray(
        xT.reshape(KT, 128, B * NB, TQ).transpose(2, 0, 1, 3))
    woT = np.ascontiguousarray(Wo.T).astype(bf16)                 # [e, f]
    woTt = np.ascontiguousarray(
        woT.reshape(KT, 128, DIM // TQ, TQ).transpose(2, 0, 1, 3))

    # RoPE tables in [d, pos] layout, tiled over batches; sin pre-signed for
    # rotate_half (rows 0:64 multiply the shifted-up half, hence negative).
    inv_freq = 1.0 / (10000.0 ** (np.arange(0, D, 2, dtype=np.float64) / D))
    t = np.arange(S, dtype=np.float64)
    freqs = np.outer(t, inv_freq)                      # [S, 64]
    emb = np.concatenate([freqs, freqs], axis=-1)      # [S, D]
    cosT = np.tile(np.cos(emb).T.astype(np.float32), (1, B)).astype(bf16)
    sinN = np.sin(emb).T.astype(np.float32)
    sinN[0:64] *= -1.0
    sinT = np.tile(sinN, (1, B)).astype(bf16)

    # single diagonal mask tile (t_local >= s_local)
    ii = np.arange(128)[:, None]
    jj = np.arange(TQ)[None, :]
    masks = (jj >= ii).astype(bf16)

    in_maps = []
    for c in range(NCORES):
        e0, e1 = c * E, (c + 1) * E
        in_maps.append({
            "xT": xTt,
            "wqT": np.ascontiguousarray(Wq[e0:e1].T).astype(bf16).reshape(KT, 128, E),
            "wkT": np.ascontiguousarray(Wk[e0:e1].T).astype(bf16).reshape(KT, 128, E),
            "wvT": np.ascontiguousarray(Wv[e0:e1].T).astype(bf16).reshape(KT, 128, E),
            "woT": woTt,
            "cosT": cosT,
            "sinT": sinT,
            "masks": masks,
        })
    return in_maps


def kernel(x, Wq, Wk, Wv, Wo, mask, _trace=False):
    from concourse.bass_utils import run_bass_kernel_spmd

    m = np.asarray(mask)
    causal = not bool(m.reshape(m.shape[-2], m.shape[-1])[0, -1])

    if causal not in _CACHE:
        _CACHE[causal] = _build(causal)
    nc = _CACHE[causal]

    in_maps = _prep_inputs(np.asarray(x), np.asarray(Wq), np.asarray(Wk),
                           np.asarray(Wv), np.asarray(Wo), causal)
    res = run_bass_kernel_spmd(nc, in_maps, core_ids=list(range(NCORES)),
                               trace=_trace)
    # core c holds rows [c*RPB, (c+1)*RPB) of every batch, b-major
    full = np.empty((B, S, DIM), np.float32)
    for c in range(NCORES):
        rc = res.results[c]["out"].reshape(B, RPB, DIM)
        full[:, c * RPB:(c + 1) * RPB, :] = rc.astype(np.float32)
    if _trace:
        return full, res
    return full


# revision 15
# speedup vs baseline: 1.0885x; 1.0885x over previous
"""Distributed causal attention (RoPE) kernel for 8 TRN2 NeuronCores.

Problem: B=4, S=2048, dim=2048, H=16 heads, D=128 head dim.
  q,k,v = x @ W{q,k,v}.T (heads), RoPE(q,k), causal softmax(q k^T/sqrt(D)) v,
  out = concat_heads @ Wo.T

Sharding: tensor-parallel over heads — 2 heads per core. Each core:
  - computes qT/kT [d, t] and v [s, e] for its 2 heads (weights pre-transposed
    host-side so every matmul operand is in its natural layout),
  - attention in "scoresT" orientation [key s on partitions, query t free]:
    exp without max-subtraction (bf16 holds e^16 fine); the softmax
    denominator comes from accumulating exp tiles on DVE (bf16 adds) and ONE
    all-ones [128,128] stationary matmul per query tile,
  - per-(batch,head) All-to-All (DRAM->DRAM) reshards attention output from
    head-shard to row-shard,
  - row-local output projection; host reassembles the row shards.

Schedule (v2): PE streams at ~0.5ns/col on this part, so the only wins are
scheduling. Warmup matmuls flip the HAM clock gate before real work arrives.
Out-projections for batches 0,1,3 are DEFERRED to an f-major tail (~100us of
matmul) that covers the last two collectives; only out_proj(b2) stays
interleaved with batch-3 attention as TensorE filler for the ACT-bound exp
chain. The ACT queue carries nothing but exps steady-state (weight/wo/at DMA
triggers live on scalar only at start/tail; x/cos/sin/ot on sync; collectives
and output stores on gpsimd). Output is written bf16 (err budget allows) to
halve store traffic.
"""

import numpy as np
import ml_dtypes

B, S, DIM = 4, 2048, 2048
H, D = 16, 128
NCORES = 8
HPC = H // NCORES            # heads per core = 2
E = HPC * D                  # per-core inner width = 256
BS = B * S                   # 8192 flattened rows
KT = DIM // 128              # 16 contraction tiles
TQ = 512                     # query tile width
NQ = S // TQ                 # 4 query tiles per (b,h)
NB = S // TQ                 # 4 x-blocks per batch
RPB = S // NCORES            # 256 output rows per core per batch
ROWS = B * RPB               # 1024 output rows per core
SCALE = 1.0 / np.sqrt(D)
WARM = 20                    # HAM warmup matmuls (N=512, ~8.5us cold: flips
                             # the clock gate and bridges to the first loads

_CACHE = {}


def _build(causal: bool):
    from concourse import bacc, tile, mybir

    f32 = mybir.dt.float32
    bf16 = mybir.dt.bfloat16
    Exp = mybir.ActivationFunctionType.Exp

    nc = bacc.Bacc(None, target_bir_lowering=False, num_devices=NCORES)

    # host layouts: xT pre-tiled [block n, ktile, 128, 512]
    xT_d = nc.dram_tensor("xT", [B * NB, KT, 128, TQ], bf16, kind="ExternalInput")
    wq_d = nc.dram_tensor("wqT", [KT, 128, E], bf16, kind="ExternalInput")
    wk_d = nc.dram_tensor("wkT", [KT, 128, E], bf16, kind="ExternalInput")
    wv_d = nc.dram_tensor("wvT", [KT, 128, E], bf16, kind="ExternalInput")
    wo_d = nc.dram_tensor("woT", [DIM // TQ, KT, 128, TQ], bf16, kind="ExternalInput")
    cos_d = nc.dram_tensor("cosT", [128, BS], bf16, kind="ExternalInput")
    sin_d = nc.dram_tensor("sinT", [128, BS], bf16, kind="ExternalInput")
    msk_d = nc.dram_tensor("masks", [128, TQ], bf16, kind="ExternalInput")
    out_d = nc.dram_tensor("out", [ROWS, DIM], bf16, kind="ExternalOutput")

    with tile.TileContext(nc) as tc:
        with (
            tc.tile_pool(name="const", bufs=1) as constp,
            tc.tile_pool(name="dram", bufs=1, space="DRAM") as dramp,
        ):
            a2a_in = [[dramp.tile([NCORES, 128, RPB], bf16, name=f"a2ai{b}h{h}")
                       for h in range(HPC)] for b in range(B)]
            a2a_out = [[dramp.tile([NCORES, 128, RPB], bf16, name=f"a2ao{b}h{h}")
                        for h in range(HPC)] for b in range(B)]

            ones_col = constp.tile([128, 128], bf16)
            nc.gpsimd.memset(ones_col[:], 1.0)

            # startup DMAs on the scalar ring (free until first exp), finest
            # first so the first projection matmul can begin after ~2 chunks.
            wq_sb = constp.tile([128, KT, E], bf16)
            wk_sb = constp.tile([128, KT, E], bf16)
            wv_sb = constp.tile([128, KT, E], bf16)
            for k in range(0, KT, 2):
                nc.scalar.dma_start(wq_sb[:, k:k + 2, :],
                                    wq_d[k:k + 2].rearrange("k p e -> p k e"))
            for k in range(0, KT, 2):
                nc.scalar.dma_start(wk_sb[:, k:k + 2, :],
                                    wk_d[k:k + 2].rearrange("k p e -> p k e"))
            for k in range(0, KT, 2):
                nc.scalar.dma_start(wv_sb[:, k:k + 2, :],
                                    wv_d[k:k + 2].rearrange("k p e -> p k e"))
            if causal:
                msk_sb = constp.tile([128, TQ], bf16)
                nc.gpsimd.dma_start(msk_sb[:], msk_d[:])

            with (
                tc.tile_pool(name="qkv", bufs=2) as qkvp,
                tc.tile_pool(name="xblk", bufs=2) as xp,
                tc.tile_pool(name="cs", bufs=2) as cp,
                tc.tile_pool(name="rope", bufs=2) as rp,
                tc.tile_pool(name="att", bufs=4) as ap,
                tc.tile_pool(name="ex", bufs=6) as exp_pool,
                tc.tile_pool(name="exa", bufs=2) as exap,
                tc.tile_pool(name="wo", bufs=2) as wop,
                tc.tile_pool(name="attin", bufs=4) as atp,
                tc.tile_pool(name="res", bufs=4) as resp,
                tc.tile_pool(name="ps1", bufs=1, space="PSUM") as pp1,
                tc.tile_pool(name="ps2", bufs=1, space="PSUM") as pp2,
            ):
                # HAM warmup: throwaway matmuls so the clock gate is at 8/8
                # by the time the first data-dependent matmul issues.
                for _ in range(WARM):
                    wps = pp1.tile([128, TQ], f32, tag="qk", bufs=2)
                    nc.tensor.matmul(wps[:], ones_col[:],
                                     msk_sb[:] if causal else wq_sb[:, 0, :],
                                     start=True, stop=True)

                def p1(b):
                    """Projections + RoPE for batch b -> qb, kb, vb."""
                    qb = qkvp.tile([128, HPC, S], bf16, tag="q", name=f"q{b}")
                    kb = qkvp.tile([128, HPC, S], bf16, tag="k", name=f"k{b}")
                    vb = qkvp.tile([128, S // 128, E], bf16, tag="v",
                                   name=f"v{b}")
                    for n in range(NB):
                        c0 = n * TQ
                        g0 = b * S + c0
                        xblk = xp.tile([128, KT, TQ], bf16, tag="xblk")
                        cos_b = cp.tile([128, TQ], bf16, tag="cos")
                        sin_b = cp.tile([128, TQ], bf16, tag="sin")
                        nc.sync.dma_start(cos_b[:], cos_d[:, g0:g0 + TQ])
                        nc.sync.dma_start(sin_b[:], sin_d[:, g0:g0 + TQ])
                        if b == 0 and n == 0:
                            # finest-grained first load so matmuls start ASAP
                            for k in range(KT):
                                nc.sync.dma_start(xblk[:, k, :], xT_d[0, k])
                        else:
                            for k in range(0, KT, 8):
                                nc.sync.dma_start(
                                    xblk[:, k:k + 8, :],
                                    xT_d[b * NB + n, k:k + 8].rearrange(
                                        "k p t -> p k t"))

                        for w_sb, dst in ((wq_sb, qb), (wk_sb, kb)):
                            for h in range(HPC):
                                ps = pp1.tile([128, TQ], f32, tag="qk", bufs=2)
                                for k in range(KT):
                                    nc.tensor.matmul(
                                        ps[:],
                                        w_sb[:, k, h * 128:(h + 1) * 128],
                                        xblk[:, k, :],
                                        start=(k == 0), stop=(k == KT - 1),
                                    )
                                t0_ = rp.tile([128, TQ], f32, tag="t0", bufs=2)
                                nc.vector.tensor_mul(t0_[0:64, :],
                                                     ps[64:128, :],
                                                     sin_b[0:64, :])
                                nc.vector.tensor_mul(t0_[64:128, :],
                                                     ps[0:64, :],
                                                     sin_b[64:128, :])
                                t1_ = rp.tile([128, TQ], f32, tag="t1", bufs=2)
                                nc.vector.tensor_mul(t1_[:], ps[:], cos_b[:])
                                nc.vector.tensor_add(dst[:, h, c0:c0 + TQ],
                                                     t0_[:], t1_[:])

                        for ss in range(TQ // 128):
                            vps = pp1.tile([128, E], f32, tag="v", bufs=1)
                            for k in range(KT):
                                nc.tensor.matmul(
                                    vps[:],
                                    xblk[:, k, ss * 128:(ss + 1) * 128],
                                    wv_sb[:, k, :],
                                    start=(k == 0), stop=(k == KT - 1),
                                )
                            nc.vector.tensor_copy(vb[:, n * 4 + ss, :], vps[:])
                    return qb, kb, vb

                def attention(b, h, qb, kb, vb, weave=None):
                    """Attention for (batch b, local head h) -> a2a_in[b][h].
                    Returns the per-qtile ot tiles (used as scheduling
                    tokens).  weave(tq, j) is called after each j-step to
                    emit a few out-projection matmuls between attention
                    matmuls — keeps TensorE dense without letting 16-matmul
                    blocks starve the ACT exp chain."""
                    ots = []
                    for tq in range(NQ):
                        t0 = tq * TQ
                        jmax = (tq + 1) * (TQ // 128) if causal else S // 128
                        av = pp2.tile([128, TQ], f32, tag="av", bufs=2)
                        exa = exap.tile([128, TQ], bf16, tag="exa", bufs=2)
                        for j in range(jmax):
                            s0 = j * 128
                            # diagonal tiles: only queries t >= s attend;
                            # compute the trapezoid [off:TQ) at reduced width
                            diag = causal and j >= jmax - 4
                            off = 128 * (j - (jmax - 4)) if diag else 0
                            w = TQ - off
                            # sc ring depth 3 (cs shares it): ACT can run up
                            # to 3 exp tiles ahead, riding out op-matmul
                            # bursts on TensorE
                            sc = pp2.tile([128, TQ], f32, tag="sc", bufs=3)
                            nc.tensor.matmul(
                                sc[:, 0:w], kb[:, h, s0:s0 + 128],
                                qb[:, h, t0 + off:t0 + TQ],
                                start=True, stop=True,
                            )
                            ex = exp_pool.tile([128, TQ], bf16, tag="ex")
                            nc.scalar.activation(ex[:, 0:w], sc[:, 0:w], Exp,
                                                 scale=float(SCALE))
                            if diag:
                                exm = exp_pool.tile([128, TQ], bf16, tag="exm",
                                                    bufs=2)
                                nc.vector.tensor_mul(exm[:, 0:w], ex[:, 0:w],
                                                     msk_sb[:, 0:w])
                                ex = exm
                            nc.tensor.matmul(
                                av[:, off:TQ], vb[:, j, h * 128:(h + 1) * 128],
                                ex[:, 0:w],
                                start=(j == 0), stop=(j == jmax - 1),
                            )
                            # softmax denominator: accumulate exp tiles in
                            # bf16 on DVE; one ones-matmul per query tile
                            if j == 0:
                                nc.vector.tensor_copy(exa[:], ex[:])
                            else:
                                nc.vector.tensor_add(exa[:, off:TQ],
                                                     exa[:, off:TQ],
                                                     ex[:, 0:w])
                            if weave is not None:
                                weave(tq, j)
                        cs = pp2.tile([128, TQ], f32, tag="sc", bufs=3)
                        nc.tensor.matmul(cs[:], ones_col[:], exa[:],
                                         start=True, stop=True)
                        # 1/colsum: approx reciprocal (~18 bits, 1 DVE op)
                        rec = ap.tile([128, TQ], f32, tag="rec", bufs=2)
                        nc.vector.reciprocal_approx_fast(rec[:], cs[:])
                        ot = ap.tile([128, TQ], bf16, tag="ot", bufs=3)
                        nc.vector.tensor_mul(ot[:], av[:], rec[:])
                        # queries [t0, t0+512) of batch b go to dest cores
                        # 2tq (first 256) and 2tq+1 (second 256)
                        nc.sync.dma_start(a2a_in[b][h][2 * tq], ot[:, 0:RPB])
                        nc.sync.dma_start(a2a_in[b][h][2 * tq + 1],
                                          ot[:, RPB:2 * RPB])
                        ots.append(ot)
                    return ots

                def a2a(b, h):
                    nc.gpsimd.collective_compute(
                        "AllToAll", mybir.AluOpType.bypass,
                        replica_groups=[list(range(NCORES))],
                        ins=[a2a_in[b][h][:].opt()],
                        outs=[a2a_out[b][h][:].opt()],
                    )

                def load_at(b, eng, dep=None, heads=(0, 1)):
                    """Load the resharded attention rows of batch b into SBUF.
                    dep (a tile) token-binds the loads so the scheduler cannot
                    place them (or dependent matmuls) ahead of work that must
                    come first."""
                    at_sb = atp.tile([128, KT, RPB], bf16, tag="at",
                                     name=f"at{b}")
                    if dep is not None:
                        nc.vector.tensor_copy(at_sb[0:1, :, 0:1],
                                              dep[0:1, 0:KT])
                    for h in heads:
                        eng.dma_start(
                            at_sb[:, h:KT:2, :],
                            a2a_out[b][h].rearrange("i p r -> p i r"))
                    return at_sb

                def op_group(b, at_sb, wo_f, f, tt, store_eng):
                    """One out-projection group: rows [tt*128,(tt+1)*128) of
                    this core's batch-b shard x output cols [f*TQ,(f+1)*TQ)."""
                    ops = pp1.tile([128, TQ], f32, tag="qk", bufs=2)
                    for ki in range(KT):
                        nc.tensor.matmul(
                            ops[:],
                            at_sb[:, ki, tt * 128:(tt + 1) * 128],
                            wo_f[:, ki, :],
                            start=(ki == 0), stop=(ki == KT - 1),
                        )
                    res = resp.tile([128, TQ], bf16, tag="res")
                    nc.vector.tensor_copy(res[:], ops[:])
                    store_eng.dma_start(
                        out_d[b * RPB + tt * 128:b * RPB + (tt + 1) * 128,
                              f * TQ:(f + 1) * TQ],
                        res[:])
                    return res

                wo_tiles = {}

                class OpWeaver:
                    """Emits one batch's out-projection a few matmuls at a
                    time so it interleaves with attention at j-granularity
                    instead of 16-matmul blocks."""

                    def __init__(self, b, at_sb):
                        self.b = b
                        self.at = at_sb
                        self.jobs = [(f, tt) for f in range(DIM // TQ)
                                     for tt in range(RPB // 128)]
                        self.ji = 0
                        self.ki = 0
                        self.ps = None

                    def step(self, n):
                        for _ in range(n):
                            if self.ji >= len(self.jobs):
                                return
                            f, tt = self.jobs[self.ji]
                            if self.ki == 0:
                                if f not in wo_tiles:
                                    wo_f = wop.tile([128, KT, TQ], bf16,
                                                    tag="wo", bufs=2)
                                    nc.sync.dma_start(
                                        wo_f[:],
                                        wo_d[f].rearrange("k p t -> p k t"))
                                    wo_tiles[f] = wo_f
                                self.ps = pp1.tile([128, TQ], f32, tag="qk",
                                                   bufs=2)
                            nc.tensor.matmul(
                                self.ps[:],
                                self.at[:, self.ki, tt * 128:(tt + 1) * 128],
                                wo_tiles[f][:, self.ki, :],
                                start=(self.ki == 0), stop=(self.ki == KT - 1),
                            )
                            self.ki += 1
                            if self.ki == KT:
                                self.ki = 0
                                self.ji += 1
                                res = resp.tile([128, TQ], bf16, tag="res")
                                nc.vector.tensor_copy(res[:], self.ps[:])
                                nc.gpsimd.dma_start(
                                    out_d[self.b * RPB + tt * 128:
                                          self.b * RPB + (tt + 1) * 128,
                                          f * TQ:(f + 1) * TQ],
                                    res[:])

                    def finish(self):
                        self.step(1 << 30)

                # ---- main pipeline ----
                prev_ot = None
                for b in range(B):
                    qb, kb, vb = p1(b)
                    if b < B - 1:
                        ot0 = attention(b, 0, qb, kb, vb)
                        a2a(b, 0)
                        prev_ot = attention(b, 1, qb, kb, vb)
                        a2a(b, 1)
                    else:
                        # out_proj(b2) woven into batch-3 attention at
                        # j-granularity: fills the ACT-bound exp chain's
                        # TensorE bubbles without starving it, and keeps the
                        # last A2A triggers early.  at2 token-bound to the end
                        # of attn(2,1) (its real dep, A2A(2,1), lands during
                        # P1(3)).
                        at2 = load_at(B - 2, nc.sync, dep=prev_ot[3])
                        weaver = OpWeaver(B - 2, at2)
                        ot0 = attention(
                            b, 0, qb, kb, vb,
                            weave=lambda tq, j: weaver.step(2) if tq >= 2
                            else None)
                        a2a(b, 0)
                        ot1 = attention(b, 1, qb, kb, vb,
                                        weave=lambda tq, j: weaver.step(2))
                        a2a(b, 1)
                        weaver.finish()

                # ---- tail: deferred out-projections cover the last two
                # collectives (~100us of matmul vs ~65us of collective) ----
                at_tiles = {0: load_at(0, nc.sync),
                            1: load_at(1, nc.sync)}
                mid_res = None
                for f in range(DIM // TQ):
                    wo_f = wop.tile([128, KT, TQ], bf16, tag="wo", bufs=2)
                    nc.scalar.dma_start(wo_f[:],
                                        wo_d[f].rearrange("k p t -> p k t"))
                    wo_tiles[f] = wo_f
                    for b2 in (0, 1):
                        for tt in range(RPB // 128):
                            eng = nc.sync if (b2 + tt) % 2 == 0 else nc.gpsimd
                            r = op_group(b2, at_tiles[b2], wo_f, f, tt, eng)
                            if f == 1 and b2 == 1 and tt == 1:
                                mid_res = r
                # batch 3: wo f2/f3 still resident from phase 1; f0/f1
                # re-stream behind them.  at3 token-bound to mid-phase-1 so
                # its matmuls strictly follow most of the cover work; head
                # halves load separately (head 0's collective lands ~30us
                # before head 1's).
                at3 = load_at(3, nc.scalar, dep=mid_res, heads=(0,))
                nc.scalar.dma_start(
                    at3[:, 1:KT:2, :],
                    a2a_out[3][1].rearrange("i p r -> p i r"))
                for f in (2, 3, 0, 1):
                    if f >= 2:
                        wo_f = wo_tiles[f]
                    else:
                        wo_f = wop.tile([128, KT, TQ], bf16, tag="wo", bufs=2)
                        nc.scalar.dma_start(
                            wo_f[:], wo_d[f].rearrange("k p t -> p k t"))
                    for tt in range(RPB // 128):
                        eng = nc.sync if tt % 2 == 0 else nc.gpsimd
                        op_group(3, at3, wo_f, f, tt, eng)

    nc.compile()
    return nc


def _prep_inputs(x, Wq, Wk, Wv, Wo, causal):
    bf16 = ml_dtypes.bfloat16
    xT = np.ascontiguousarray(x.reshape(BS, DIM).T).astype(bf16)  # [dim, BS]
    # pre-tile: [block n, ktile, 128, 512]
    xTt = np.ascontiguousarray(
        xT.reshape(KT, 128, B * NB, TQ).transpose(2, 0, 1, 3))
    woT = np.ascontiguousarray(Wo.T).astype(bf16)                 # [e, f]
    woTt = np.ascontiguousarray(
        woT.reshape(KT, 128, DIM // TQ, TQ).transpose(2, 0, 1, 3))

    # RoPE tables in [d, pos] layout, tiled over batches; sin pre-signed for
    # rotate_half (rows 0:64 multiply the shifted-up half, hence negative).
    inv_freq = 1.0 / (10000.0 ** (np.arange(0, D, 2, dtype=np.float64) / D))
    t = np.arange(S, dtype=np.float64)
    freqs = np.outer(t, inv_freq)                      # [S, 64]
    emb = np.concatenate([freqs, freqs], axis=-1)      # [S, D]
    cosT = np.tile(np.cos(emb).T.astype(np.float32), (1, B)).astype(bf16)
    sinN = np.sin(emb).T.astype(np.float32)
    sinN[0:64] *= -1.0
    sinT = np.tile(sinN, (1, B)).astype(bf16)

    # single diagonal mask tile (t_local >= s_local)
    ii = np.arange(128)[:, None]
    jj = np.arange(TQ)[None, :]
    masks = (jj >= ii).astype(bf16)

    in_maps = []
    for c in range(NCORES):
        e0, e1 = c * E, (c + 1) * E
        in_maps.append({
            "xT": xTt,
            "wqT": np.ascontiguousarray(Wq[e0:e1].T).astype(bf16).reshape(KT, 128, E),
            "wkT": np.ascontiguousarray(Wk[e0:e1].T).astype(bf16).reshape(KT, 128, E),
            "wvT": np.ascontiguousarray(Wv[e0:e1].T).astype(bf16).reshape(KT, 128, E),
            "woT": woTt,
            "cosT": cosT,
            "sinT": sinT,
            "masks": masks,
        })
    return in_maps


def kernel(x, Wq, Wk, Wv, Wo, mask, _trace=False):
    from concourse.bass_utils import run_bass_kernel_spmd

    m = np.asarray(mask)
    causal = not bool(m.reshape(m.shape[-2], m.shape[-1])[0, -1])

    if causal not in _CACHE:
        _CACHE[causal] = _build(causal)
    nc = _CACHE[causal]

    in_maps = _prep_inputs(np.asarray(x), np.asarray(Wq), np.asarray(Wk),
                           np.asarray(Wv), np.asarray(Wo), causal)
    res = run_bass_kernel_spmd(nc, in_maps, core_ids=list(range(NCORES)),
                               trace=_trace)
    # core c holds rows [c*RPB, (c+1)*RPB) of every batch, b-major
    full = np.empty((B, S, DIM), np.float32)
    for c in range(NCORES):
        rc = res.results[c]["out"].reshape(B, RPB, DIM)
        full[:, c * RPB:(c + 1) * RPB, :] = rc.astype(np.float32)
    if _trace:
        return full, res
    return full


# revision 20
# speedup vs baseline: 1.0932x; 1.0043x over previous
"""Distributed causal attention (RoPE) kernel for 8 TRN2 NeuronCores.

Problem: B=4, S=2048, dim=2048, H=16 heads, D=128 head dim.
  q,k,v = x @ W{q,k,v}.T (heads), RoPE(q,k), causal softmax(q k^T/sqrt(D)) v,
  out = concat_heads @ Wo.T

Sharding: tensor-parallel over heads — 2 heads per core. Each core:
  - computes qT/kT [d, t] and v [s, e] for its 2 heads (weights pre-transposed
    host-side so every matmul operand is in its natural layout),
  - attention in "scoresT" orientation [key s on partitions, query t free]:
    exp without max-subtraction (bf16 holds e^16 fine); the softmax
    denominator comes from accumulating exp tiles on DVE (bf16 adds) and ONE
    all-ones [128,128] stationary matmul per query tile,
  - per-(batch,head) All-to-All (DRAM->DRAM) reshards attention output from
    head-shard to row-shard,
  - row-local output projection; host reassembles the row shards.

Schedule (v2): PE streams at ~0.5ns/col on this part, so the only wins are
scheduling. Warmup matmuls flip the HAM clock gate before real work arrives.
Out-projections for batches 0,1,3 are DEFERRED to an f-major tail (~100us of
matmul) that covers the last two collectives; only out_proj(b2) stays
interleaved with batch-3 attention as TensorE filler for the ACT-bound exp
chain. The ACT queue carries nothing but exps steady-state (weight/wo/at DMA
triggers live on scalar only at start/tail; x/cos/sin/ot on sync; collectives
and output stores on gpsimd). Output is written bf16 (err budget allows) to
halve store traffic.
"""

import numpy as np
import ml_dtypes

B, S, DIM = 4, 2048, 2048
H, D = 16, 128
NCORES = 8
HPC = H // NCORES            # heads per core = 2
E = HPC * D                  # per-core inner width = 256
BS = B * S                   # 8192 flattened rows
KT = DIM // 128              # 16 contraction tiles
TQ = 512                     # query tile width
NQ = S // TQ                 # 4 query tiles per (b,h)
NB = S // TQ                 # 4 x-blocks per batch
RPB = S // NCORES            # 256 output rows per core per batch
ROWS = B * RPB               # 1024 output rows per core
SCALE = 1.0 / np.sqrt(D)
WARM = 26                    # HAM warmup matmuls (N=512, ~11us cold: flips
                             # the clock gate and bridges to the first loads

_CACHE = {}


def _build(causal: bool):
    from concourse import bacc, tile, mybir

    f32 = mybir.dt.float32
    bf16 = mybir.dt.bfloat16
    Exp = mybir.ActivationFunctionType.Exp

    nc = bacc.Bacc(None, target_bir_lowering=False, num_devices=NCORES)

    # host layouts: xT pre-tiled [block n, ktile, 128, 512]
    xT_d = nc.dram_tensor("xT", [B * NB, KT, 128, TQ], bf16, kind="ExternalInput")
    wq_d = nc.dram_tensor("wqT", [KT, 128, E], bf16, kind="ExternalInput")
    wk_d = nc.dram_tensor("wkT", [KT, 128, E], bf16, kind="ExternalInput")
    wv_d = nc.dram_tensor("wvT", [KT, 128, E], bf16, kind="ExternalInput")
    wo_d = nc.dram_tensor("woT", [DIM // TQ, KT, 128, TQ], bf16, kind="ExternalInput")
    cos_d = nc.dram_tensor("cosT", [128, BS], bf16, kind="ExternalInput")
    sin_d = nc.dram_tensor("sinT", [128, BS], bf16, kind="ExternalInput")
    msk_d = nc.dram_tensor("masks", [128, TQ], bf16, kind="ExternalInput")
    out_d = nc.dram_tensor("out", [ROWS, DIM], bf16, kind="ExternalOutput")

    with tile.TileContext(nc) as tc:
        with (
            tc.tile_pool(name="const", bufs=1) as constp,
            tc.tile_pool(name="dram", bufs=1, space="DRAM") as dramp,
        ):
            a2a_in = [[dramp.tile([NCORES, 128, RPB], bf16, name=f"a2ai{b}h{h}")
                       for h in range(HPC)] for b in range(B)]
            a2a_out = [[dramp.tile([NCORES, 128, RPB], bf16, name=f"a2ao{b}h{h}")
                        for h in range(HPC)] for b in range(B)]

            ones_col = constp.tile([128, 128], bf16)
            nc.gpsimd.memset(ones_col[:], 1.0)

            # startup DMAs on the scalar ring (free until first exp), finest
            # first so the first projection matmul can begin after ~2 chunks.
            wq_sb = constp.tile([128, KT, E], bf16)
            wk_sb = constp.tile([128, KT, E], bf16)
            wv_sb = constp.tile([128, KT, E], bf16)
            for k in range(0, KT, 2):
                nc.scalar.dma_start(wq_sb[:, k:k + 2, :],
                                    wq_d[k:k + 2].rearrange("k p e -> p k e"))
            for k in range(0, KT, 2):
                nc.scalar.dma_start(wk_sb[:, k:k + 2, :],
                                    wk_d[k:k + 2].rearrange("k p e -> p k e"))
            for k in range(0, KT, 2):
                nc.scalar.dma_start(wv_sb[:, k:k + 2, :],
                                    wv_d[k:k + 2].rearrange("k p e -> p k e"))
            if causal:
                msk_sb = constp.tile([128, TQ], bf16)
                nc.gpsimd.dma_start(msk_sb[:], msk_d[:])

            with (
                tc.tile_pool(name="qkv", bufs=2) as qkvp,
                tc.tile_pool(name="xblk", bufs=2) as xp,
                tc.tile_pool(name="cs", bufs=2) as cp,
                tc.tile_pool(name="rope", bufs=2) as rp,
                tc.tile_pool(name="att", bufs=4) as ap,
                tc.tile_pool(name="ex", bufs=6) as exp_pool,
                tc.tile_pool(name="exa", bufs=2) as exap,
                tc.tile_pool(name="wo", bufs=2) as wop,
                tc.tile_pool(name="attin", bufs=4) as atp,
                tc.tile_pool(name="res", bufs=4) as resp,
                tc.tile_pool(name="ps1", bufs=1, space="PSUM") as pp1,
                tc.tile_pool(name="ps2", bufs=1, space="PSUM") as pp2,
            ):
                # HAM warmup: throwaway matmuls so the clock gate is at 8/8
                # by the time the first data-dependent matmul issues.
                for _ in range(WARM):
                    wps = pp1.tile([128, TQ], f32, tag="qk", bufs=2)
                    nc.tensor.matmul(wps[:], ones_col[:],
                                     msk_sb[:] if causal else wq_sb[:, 0, :],
                                     start=True, stop=True)

                def p1(b):
                    """Projections + RoPE for batch b -> qb, kb, vb.
                    Block loads are emitted one block ahead of their matmuls
                    so every x stream gets a full group-duration head start."""
                    qb = qkvp.tile([128, HPC, S], bf16, tag="q", name=f"q{b}")
                    kb = qkvp.tile([128, HPC, S], bf16, tag="k", name=f"k{b}")
                    vb = qkvp.tile([128, S // 128, E], bf16, tag="v",
                                   name=f"v{b}")

                    def load_block(n):
                        g0 = b * S + n * TQ
                        xblk = xp.tile([128, KT, TQ], bf16, tag="xblk")
                        cos_b = cp.tile([128, TQ], bf16, tag="cos")
                        sin_b = cp.tile([128, TQ], bf16, tag="sin")
                        nc.sync.dma_start(cos_b[:], cos_d[:, g0:g0 + TQ])
                        nc.sync.dma_start(sin_b[:], sin_d[:, g0:g0 + TQ])
                        if b == 0 and n == 0:
                            # finest-grained first load so matmuls start ASAP
                            for k in range(KT):
                                nc.sync.dma_start(xblk[:, k, :], xT_d[0, k])
                        else:
                            for k in range(0, KT, 8):
                                nc.sync.dma_start(
                                    xblk[:, k:k + 8, :],
                                    xT_d[b * NB + n, k:k + 8].rearrange(
                                        "k p t -> p k t"))
                        return xblk, cos_b, sin_b

                    pending = load_block(0)
                    for n in range(NB):
                        c0 = n * TQ
                        xblk, cos_b, sin_b = pending
                        if n + 1 < NB:
                            pending = load_block(n + 1)

                        for w_sb, dst in ((wq_sb, qb), (wk_sb, kb)):
                            for h in range(HPC):
                                ps = pp1.tile([128, TQ], f32, tag="qk", bufs=2)
                                for k in range(KT):
                                    nc.tensor.matmul(
                                        ps[:],
                                        w_sb[:, k, h * 128:(h + 1) * 128],
                                        xblk[:, k, :],
                                        start=(k == 0), stop=(k == KT - 1),
                                    )
                                t0_ = rp.tile([128, TQ], f32, tag="t0", bufs=2)
                                nc.vector.tensor_mul(t0_[0:64, :],
                                                     ps[64:128, :],
                                                     sin_b[0:64, :])
                                nc.vector.tensor_mul(t0_[64:128, :],
                                                     ps[0:64, :],
                                                     sin_b[64:128, :])
                                t1_ = rp.tile([128, TQ], f32, tag="t1", bufs=2)
                                nc.vector.tensor_mul(t1_[:], ps[:], cos_b[:])
                                nc.vector.tensor_add(dst[:, h, c0:c0 + TQ],
                                                     t0_[:], t1_[:])

                        for ss in range(TQ // 128):
                            vps = pp1.tile([128, E], f32, tag="v", bufs=1)
                            for k in range(KT):
                                nc.tensor.matmul(
                                    vps[:],
                                    xblk[:, k, ss * 128:(ss + 1) * 128],
                                    wv_sb[:, k, :],
                                    start=(k == 0), stop=(k == KT - 1),
                                )
                            nc.vector.tensor_copy(vb[:, n * 4 + ss, :], vps[:])
                    return qb, kb, vb

                def attention(b, h, qb, kb, vb, weave=None):
                    """Attention for (batch b, local head h) -> a2a_in[b][h].
                    Returns the per-qtile ot tiles (used as scheduling
                    tokens).  weave(tq, j) is called after each j-step to
                    emit a few out-projection matmuls between attention
                    matmuls — keeps TensorE dense without letting 16-matmul
                    blocks starve the ACT exp chain."""
                    ots = []
                    for tq in range(NQ):
                        t0 = tq * TQ
                        jmax = (tq + 1) * (TQ // 128) if causal else S // 128
                        av = pp2.tile([128, TQ], f32, tag="av", bufs=2)
                        exa = exap.tile([128, TQ], bf16, tag="exa", bufs=2)
                        for j in range(jmax):
                            s0 = j * 128
                            # diagonal tiles: only queries t >= s attend;
                            # compute the trapezoid [off:TQ) at reduced width
                            diag = causal and j >= jmax - 4
                            off = 128 * (j - (jmax - 4)) if diag else 0
                            w = TQ - off
                            # sc ring depth 3 (cs shares it): ACT can run up
                            # to 3 exp tiles ahead, riding out op-matmul
                            # bursts on TensorE
                            sc = pp2.tile([128, TQ], f32, tag="sc", bufs=3)
                            nc.tensor.matmul(
                                sc[:, 0:w], kb[:, h, s0:s0 + 128],
                                qb[:, h, t0 + off:t0 + TQ],
                                start=True, stop=True,
                            )
                            ex = exp_pool.tile([128, TQ], bf16, tag="ex",
                                               bufs=8)
                            nc.scalar.activation(ex[:, 0:w], sc[:, 0:w], Exp,
                                                 scale=float(SCALE))
                            if diag:
                                exm = exp_pool.tile([128, TQ], bf16, tag="exm",
                                                    bufs=2)
                                nc.vector.tensor_mul(exm[:, 0:w], ex[:, 0:w],
                                                     msk_sb[:, 0:w])
                                ex = exm
                            nc.tensor.matmul(
                                av[:, off:TQ], vb[:, j, h * 128:(h + 1) * 128],
                                ex[:, 0:w],
                                start=(j == 0), stop=(j == jmax - 1),
                            )
                            # softmax denominator: accumulate exp tiles in
                            # bf16 on DVE; one ones-matmul per query tile
                            if j == 0:
                                nc.vector.tensor_copy(exa[:], ex[:])
                            else:
                                nc.vector.tensor_add(exa[:, off:TQ],
                                                     exa[:, off:TQ],
                                                     ex[:, 0:w])
                            if weave is not None:
                                weave(tq, j)
                        cs = pp2.tile([128, TQ], f32, tag="sc", bufs=3)
                        nc.tensor.matmul(cs[:], ones_col[:], exa[:],
                                         start=True, stop=True)
                        # 1/colsum: approx reciprocal (~18 bits, 1 DVE op)
                        rec = ap.tile([128, TQ], f32, tag="rec", bufs=2)
                        nc.vector.reciprocal_approx_fast(rec[:], cs[:])
                        ot = ap.tile([128, TQ], bf16, tag="ot", bufs=3)
                        nc.vector.tensor_mul(ot[:], av[:], rec[:])
                        # queries [t0, t0+512) of batch b go to dest cores
                        # 2tq (first 256) and 2tq+1 (second 256)
                        nc.sync.dma_start(a2a_in[b][h][2 * tq], ot[:, 0:RPB])
                        nc.sync.dma_start(a2a_in[b][h][2 * tq + 1],
                                          ot[:, RPB:2 * RPB])
                        ots.append(ot)
                    return ots

                def a2a(b, h):
                    nc.gpsimd.collective_compute(
                        "AllToAll", mybir.AluOpType.bypass,
                        replica_groups=[list(range(NCORES))],
                        ins=[a2a_in[b][h][:].opt()],
                        outs=[a2a_out[b][h][:].opt()],
                    )

                def load_at(b, eng, dep=None, heads=(0, 1)):
                    """Load the resharded attention rows of batch b into SBUF.
                    dep (a tile) token-binds the loads so the scheduler cannot
                    place them (or dependent matmuls) ahead of work that must
                    come first."""
                    at_sb = atp.tile([128, KT, RPB], bf16, tag="at",
                                     name=f"at{b}")
                    if dep is not None:
                        nc.vector.tensor_copy(at_sb[0:1, :, 0:1],
                                              dep[0:1, 0:KT])
                    for h in heads:
                        eng.dma_start(
                            at_sb[:, h:KT:2, :],
                            a2a_out[b][h].rearrange("i p r -> p i r"))
                    return at_sb

                def op_group(b, at_sb, wo_f, f, tt, store_eng, split=False):
                    """One out-projection group: rows [tt*128,(tt+1)*128) of
                    this core's batch-b shard x output cols [f*TQ,(f+1)*TQ).
                    split=True stores two halves on separate queues (for the
                    kernel's last groups, where store time is exposed)."""
                    ops = pp1.tile([128, TQ], f32, tag="qk", bufs=2)
                    for ki in range(KT):
                        nc.tensor.matmul(
                            ops[:],
                            at_sb[:, ki, tt * 128:(tt + 1) * 128],
                            wo_f[:, ki, :],
                            start=(ki == 0), stop=(ki == KT - 1),
                        )
                    res = resp.tile([128, TQ], bf16, tag="res")
                    rows = out_d[b * RPB + tt * 128:b * RPB + (tt + 1) * 128,
                                 f * TQ:(f + 1) * TQ]
                    if split:
                        nc.vector.tensor_copy(res[:, 0:TQ // 2],
                                              ops[:, 0:TQ // 2])
                        nc.sync.dma_start(rows[:, 0:TQ // 2],
                                          res[:, 0:TQ // 2])
                        nc.vector.tensor_copy(res[:, TQ // 2:], ops[:, TQ // 2:])
                        nc.gpsimd.dma_start(rows[:, TQ // 2:], res[:, TQ // 2:])
                    else:
                        nc.vector.tensor_copy(res[:], ops[:])
                        store_eng.dma_start(rows, res[:])
                    return res

                wo_tiles = {}

                class OpWeaver:
                    """Emits one batch's out-projection a few matmuls at a
                    time so it interleaves with attention at j-granularity
                    instead of 16-matmul blocks."""

                    def __init__(self, b, at_sb):
                        self.b = b
                        self.at = at_sb
                        self.jobs = [(f, tt) for f in range(DIM // TQ)
                                     for tt in range(RPB // 128)]
                        self.ji = 0
                        self.ki = 0
                        self.ps = None

                    def step(self, n):
                        for _ in range(n):
                            if self.ji >= len(self.jobs):
                                return
                            f, tt = self.jobs[self.ji]
                            if self.ki == 0:
                                if f not in wo_tiles:
                                    wo_f = wop.tile([128, KT, TQ], bf16,
                                                    tag="wo", bufs=2)
                                    nc.sync.dma_start(
                                        wo_f[:],
                                        wo_d[f].rearrange("k p t -> p k t"))
                                    wo_tiles[f] = wo_f
                                self.ps = pp1.tile([128, TQ], f32, tag="qk",
                                                   bufs=2)
                            nc.tensor.matmul(
                                self.ps[:],
                                self.at[:, self.ki, tt * 128:(tt + 1) * 128],
                                wo_tiles[f][:, self.ki, :],
                                start=(self.ki == 0), stop=(self.ki == KT - 1),
                            )
                            self.ki += 1
                            if self.ki == KT:
                                self.ki = 0
                                self.ji += 1
                                res = resp.tile([128, TQ], bf16, tag="res")
                                nc.vector.tensor_copy(res[:], self.ps[:])
                                nc.gpsimd.dma_start(
                                    out_d[self.b * RPB + tt * 128:
                                          self.b * RPB + (tt + 1) * 128,
                                          f * TQ:(f + 1) * TQ],
                                    res[:])

                    def finish(self):
                        self.step(1 << 30)

                # ---- main pipeline ----
                prev_ot = None
                for b in range(B):
                    qb, kb, vb = p1(b)
                    if b < B - 1:
                        ot0 = attention(b, 0, qb, kb, vb)
                        a2a(b, 0)
                        prev_ot = attention(b, 1, qb, kb, vb)
                        a2a(b, 1)
                    else:
                        # out_proj(b2) woven into batch-3 attention at
                        # j-granularity: fills the ACT-bound exp chain's
                        # TensorE bubbles without starving it, and keeps the
                        # last A2A triggers early.  at2 token-bound to the end
                        # of attn(2,1) (its real dep, A2A(2,1), lands during
                        # P1(3)).
                        at2 = load_at(B - 2, nc.sync, dep=prev_ot[3])
                        weaver = OpWeaver(B - 2, at2)
                        ot0 = attention(
                            b, 0, qb, kb, vb,
                            weave=lambda tq, j: weaver.step(2) if tq >= 2
                            else None)
                        a2a(b, 0)
                        ot1 = attention(b, 1, qb, kb, vb,
                                        weave=lambda tq, j: weaver.step(2))
                        a2a(b, 1)
                        weaver.finish()

                # ---- tail: deferred out-projections cover the last two
                # collectives (~100us of matmul vs ~65us of collective) ----
                at_tiles = {0: load_at(0, nc.sync),
                            1: load_at(1, nc.sync)}
                mid_res = None
                for f in range(DIM // TQ):
                    wo_f = wop.tile([128, KT, TQ], bf16, tag="wo", bufs=2)
                    nc.scalar.dma_start(wo_f[:],
                                        wo_d[f].rearrange("k p t -> p k t"))
                    wo_tiles[f] = wo_f
                    for b2 in (0, 1):
                        for tt in range(RPB // 128):
                            eng = nc.sync if (b2 + tt) % 2 == 0 else nc.gpsimd
                            r = op_group(b2, at_tiles[b2], wo_f, f, tt, eng)
                            if f == 1 and b2 == 1 and tt == 1:
                                mid_res = r
                # batch 3: wo f2/f3 still resident from phase 1; f0/f1
                # re-stream behind them.  at3 token-bound to mid-phase-1 so
                # its matmuls strictly follow most of the cover work; head
                # halves load separately (head 0's collective lands ~30us
                # before head 1's).
                at3 = load_at(3, nc.scalar, dep=mid_res, heads=(0,))
                nc.scalar.dma_start(
                    at3[:, 1:KT:2, :],
                    a2a_out[3][1].rearrange("i p r -> p i r"))
                for f in (2, 3, 0, 1):
                    if f >= 2:
                        wo_f = wo_tiles[f]
                    else:
                        # chunked reload: first matmuls start after the first
                        # 512KB chunk instead of the full 2MB
                        wo_f = wop.tile([128, KT, TQ], bf16, tag="wo", bufs=2)
                        for k in range(0, KT, 4):
                            nc.scalar.dma_start(
                                wo_f[:, k:k + 4, :],
                                wo_d[f, k:k + 4].rearrange("k p t -> p k t"))
                    for tt in range(RPB // 128):
                        eng = nc.sync if tt % 2 == 0 else nc.gpsimd
                        op_group(3, at3, wo_f, f, tt, eng,
                                 split=(f == 1))

    nc.compile()
    return nc


def _prep_inputs(x, Wq, Wk, Wv, Wo, causal):
    bf16 = ml_dtypes.bfloat16
    xT = np.ascontiguousarray(x.reshape(BS, DIM).T).astype(bf16)  # [dim, BS]
    # pre-tile: [block n, ktile, 128, 512]
    xTt = np.ascontiguousarray(
        xT.reshape(KT, 128, B * NB, TQ).transpose(2, 0, 1, 3))
    woT = np.ascontiguousarray(Wo.T).astype(bf16)                 # [e, f]
    woTt = np.ascontiguousarray(
        woT.reshape(KT, 128, DIM // TQ, TQ).transpose(2, 0, 1, 3))

    # RoPE tables in [d, pos] layout, tiled over batches; sin pre-signed for
    # rotate_half (rows 0:64 multiply the shifted-up half, hence negative).
    inv_freq = 1.0 / (10000.0 ** (np.arange(0, D, 2, dtype=np.float64) / D))
    t = np.arange(S, dtype=np.float64)
    freqs = np.outer(t, inv_freq)                      # [S, 64]
    emb = np.concatenate([freqs, freqs], axis=-1)      # [S, D]
    cosT = np.tile(np.cos(emb).T.astype(np.float32), (1, B)).astype(bf16)
    sinN = np.sin(emb).T.astype(np.float32)
    sinN[0:64] *= -1.0
    sinT = np.tile(sinN, (1, B)).astype(bf16)

    # single diagonal mask tile (t_local >= s_local)
    ii = np.arange(128)[:, None]
    jj = np.arange(TQ)[None, :]
    masks = (jj >= ii).astype(bf16)

    in_maps = []
    for c in range(NCORES):
        e0, e1 = c * E, (c + 1) * E
        in_maps.append({
            "xT": xTt,
            "wqT": np.ascontiguousarray(Wq[e0:e1].T).astype(bf16).reshape(KT, 128, E),
            "wkT": np.ascontiguousarray(Wk[e0:e1].T).astype(bf16).reshape(KT, 128, E),
            "wvT": np.ascontiguousarray(Wv[e0:e1].T).astype(bf16).reshape(KT, 128, E),
            "woT": woTt,
            "cosT": cosT,
            "sinT": sinT,
            "masks": masks,
        })
    return in_maps


def kernel(x, Wq, Wk, Wv, Wo, mask, _trace=False):
    from concourse.bass_utils import run_bass_kernel_spmd

    m = np.asarray(mask)
    causal = not bool(m.reshape(m.shape[-2], m.shape[-1])[0, -1])

    if causal not in _CACHE:
        _CACHE[causal] = _build(causal)
    nc = _CACHE[causal]

    in_maps = _prep_inputs(np.asarray(x), np.asarray(Wq), np.asarray(Wk),
                           np.asarray(Wv), np.asarray(Wo), causal)
    res = run_bass_kernel_spmd(nc, in_maps, core_ids=list(range(NCORES)),
                               trace=_trace)
    # core c holds rows [c*RPB, (c+1)*RPB) of every batch, b-major
    full = np.empty((B, S, DIM), np.float32)
    for c in range(NCORES):
        rc = res.results[c]["out"].reshape(B, RPB, DIM)
        full[:, c * RPB:(c + 1) * RPB, :] = rc.astype(np.float32)
    if _trace:
        return full, res
    return full
